# revision 1
# baseline (speedup 1.0000x reference)
"""ClsAttention pooling kernel for 8 TRN2 NeuronCores.

reference:
    att_logits = einsum('bch,nc->bnh', feats, W)      # [B, N, HW]
    att_maps   = softmax(att_logits, axis=2)          # softmax over HW
    cls_feats  = einsum('bnh,bch->bnc', att_maps, feats)

Strategy (data-parallel over batch, 4 items per core):
  - One HBM pass over feats. Each item's [C, HW] slab is DMA-loaded with an
    f32->fp16 cast (SWDGE), then transposed on-chip to [HW, C] via the DMA
    xbar (2-byte dtype requirement is why we compute in fp16).
  - mm1 uses the natural-layout feats chunk as the PE stationary operand so
    logits^T comes out directly in [h, n] layout; exp runs on ScalarE out of
    PSUM. Softmax normalization is deferred: cls = (E @ feats^T) / Z with
    Z accumulated by an extra N=1 matmul against a ones vector (exp without
    max-subtraction is safe: logits ~ N(0,1)).
  - All matmul accumulation in fp32 PSUM; final normalize in fp32.
"""

import numpy as np

import concourse.bass as bass
import concourse.mybir as mybir
import concourse.tile as tile
from concourse import bacc
from concourse.bass_utils import run_bass_kernel_spmd

B, C, HW, NCLS = 32, 512, 4096, 80
NCORES = 8
BPC = B // NCORES  # batch items per core
CCH = C // 128     # contraction chunks for mm1
HCH = HW // 128    # h chunks (contraction for mm2)
HG = 4             # h-chunks packed per PSUM logits tile
CDT = mybir.dt.float16
F32 = mybir.dt.float32

_cached_nc = None


def _build():
    global _cached_nc
    if _cached_nc is not None:
        return _cached_nc
    nc = bacc.Bacc("TRN2", target_bir_lowering=False, debug=False)
    feats = nc.dram_tensor("feats", [BPC, C, HW], F32, kind="ExternalInput")
    wt = nc.dram_tensor("wt", [C, NCLS], F32, kind="ExternalInput")
    out = nc.dram_tensor("out", [BPC, NCLS, C], F32, kind="ExternalOutput")

    with tile.TileContext(nc) as tc:
        with (
            tc.tile_pool(name="singles", bufs=1) as singles,
            tc.tile_pool(name="fpool", bufs=2) as fpool,
            tc.tile_pool(name="tpool", bufs=2) as tpool,
            tc.tile_pool(name="epool", bufs=2) as epool,
            tc.tile_pool(name="opool", bufs=2) as opool,
            tc.tile_pool(name="plp", bufs=2, space="PSUM") as plp,
            tc.tile_pool(name="pup", bufs=2, space="PSUM") as pup,
            tc.tile_pool(name="pzp", bufs=2, space="PSUM") as pzp,
        ):
            wt_sb = singles.tile([128, CCH, NCLS], CDT)
            for ci in range(CCH):
                nc.gpsimd.dma_start(
                    out=wt_sb[:, ci, :], in_=wt[128 * ci : 128 * (ci + 1), :]
                )
            ones = singles.tile([128, 1], CDT)
            nc.vector.memset(ones, 1.0)

            for b in range(BPC):
                # load + cast feats[b] to fp16, natural [c, h] layout
                fb = fpool.tile([128, CCH, HW], CDT)
                for ci in range(CCH):
                    nc.gpsimd.dma_start(
                        out=fb[:, ci, :], in_=feats[b, 128 * ci : 128 * (ci + 1), :]
                    )
                # on-chip transpose -> ftT[p, hj, c] = feats^T[hj*128+p, c]
                ftT = tpool.tile([128, HCH, C], CDT)
                for ci in range(CCH):
                    nc.sync.dma_start_transpose(
                        out=ftT[:, :, 128 * ci : 128 * (ci + 1)], in_=fb[:, ci, :]
                    )
                # mm1: logits^T chunks [128h, NCLS] + exp -> eT
                eT = epool.tile([128, HCH, NCLS], CDT)
                for hg in range(HCH // HG):
                    pl = plp.tile([128, HG, NCLS], F32)
                    for t in range(HG):
                        hj = hg * HG + t
                        for ci in range(CCH):
                            nc.tensor.matmul(
                                pl[:, t, :],
                                lhsT=fb[:, ci, bass.ts(hj, 128)],
                                rhs=wt_sb[:, ci, :],
                                start=(ci == 0),
                                stop=(ci == CCH - 1),
                            )
                    nc.scalar.activation(
                        out=eT[:, hg * HG : (hg + 1) * HG, :],
                        in_=pl,
                        func=mybir.ActivationFunctionType.Exp,
                    )
                # mm2: U = E @ feats^T, Z = E @ 1 (accumulate over h chunks)
                pu = pup.tile([NCLS, C], F32)
                pz = pzp.tile([NCLS, 1], F32)
                for hj in range(HCH):
                    nc.tensor.matmul(
                        pu,
                        lhsT=eT[:, hj, :],
                        rhs=ftT[:, hj, :],
                        start=(hj == 0),
                        stop=(hj == HCH - 1),
                    )
                    nc.tensor.matmul(
                        pz,
                        lhsT=eT[:, hj, :],
                        rhs=ones,
                        start=(hj == 0),
                        stop=(hj == HCH - 1),
                    )
                # cls = U / Z
                zr = opool.tile([NCLS, 1], F32)
                nc.vector.reciprocal(zr, pz)
                ob = opool.tile([NCLS, C], F32)
                nc.vector.tensor_scalar_mul(ob, pu, zr)
                nc.sync.dma_start(out=out[b], in_=ob)

    nc.compile()
    _cached_nc = nc
    return nc


def kernel(feats: np.ndarray, W: np.ndarray, **run_kwargs) -> np.ndarray:
    nc = _build()
    feats = np.ascontiguousarray(np.asarray(feats), dtype=np.float32)
    wt = np.ascontiguousarray(np.asarray(W, dtype=np.float32).T)
    in_maps = [
        {"feats": np.ascontiguousarray(feats[i * BPC : (i + 1) * BPC]), "wt": wt}
        for i in range(NCORES)
    ]
    res = run_bass_kernel_spmd(nc, in_maps, list(range(NCORES)), **run_kwargs)
    out = np.concatenate([r["out"] for r in res.results], axis=0)
    if run_kwargs:
        kernel.last_results = res
    return np.asarray(out, dtype=np.float32)
